# revision 5
# baseline (speedup 1.0000x reference)
"""Query-chunk-parallel MultiHeadAttention kernel for 8 Trainium2 cores.

Problem: B=2, S=2048, D=512, H=8, per-head full-width projections.

Sharding: the B*S=4096 query rows split into 8 chunks of 512; chunk c
-> core c (b = c//4). Each core computes ALL 8 heads for its 512 query
rows and writes its own [512, D] slice of the final output: ZERO
collectives (the head-parallel variant spent ~70us in an exposed
ReduceScatter/AllGather tail plus a saturated cc stream).

Math restructuring (inherited from the verified head-parallel kernel):
  - softmax row-equivalences drop the K bias bk entirely; the V bias bv
    reduces to a constant row c = sum_h bv[h] @ Wo_h + bo added on the
    host at the end.
  - Host-fused weights (weight-weight products only):
      M_h = (Wq[h]/sqrt(D)) @ Wk[h]^T   so scores = q M_h k^T
      u_h = (bq[h]/sqrt(D)) @ Wk[h]^T   per-partition bias on QM^T
      W2_h = Wv[h] @ Wo_h               so out += (attn @ v) @ W2_h / den
    This removes the on-device K and V projections completely.
  - No softmax max-subtraction: score std ~0.33, |scores| < ~2.5.

Dataflow per head h (on this core's 512-row query chunk):
  QM^T[d2,m] = M_h^T q^T, +u, *64 -> fp8   (8 MM, fp8 DoubleRow)
  sT[k,m]    = k8 QM8                      (32 MM, fp8 DoubleRow)
               -> exp(ps/1024) on ACT -> PT (bf16)
  AT[d,m]    = v^T P (bf16)                (64 MM), den = DVE adds
  acc[m,e]  += (AT^T W2_h) / den_h         (16 MM + 2 DVE)
After head 7: acc (f32) -> bf16 -> DMA to the core's out slice.

Perf notes (from NTFF traces of the head-parallel ancestor):
  - Every 128-part x 512-free matmul costs ~263ns regardless of dtype
    (512 rows at the sustained-clock rate); fp8 DoubleRow wins by
    contracting 256 rows/MM, i.e. halved MM count, not faster MMs.
    960 MMs/core ~= 253us is the PE floor at these precisions.
  - Software pipelining: QM+scores of head h+1 interleave into the
    AV/out-projection of head h on the PE (ratio 2:1) so the exp (ACT)
    latency and QM->fp8 casts (DVE) never pace the tensor engine.
  - PE warm-up dummy matmuls keep the clock ramped through the initial
    DMA wait; loads are ordered first-use-first (wm_h0+q, k, v, ...).
"""
import os
import sys

sys.path.insert(0, "/opt/trn_rl_repo")
sys.path.insert(0, "/root/.axon_site")

import numpy as np

import concourse.bacc as bacc
import concourse.mybir as mybir
from concourse.tile import TileContext
from concourse import bass_utils

P = 128
B, S, D, H = 2, 2048, 512, 8
NCORES = 8
MC = 4               # query chunks per batch; B*MC == NCORES
CH = S // MC         # 512 query rows per core
DT = D // P          # 4 feature tiles
KT = S // P          # 16 k tiles (full batch seq per core)
F32 = mybir.dt.float32
BF16 = mybir.dt.bfloat16
FP8 = mybir.dt.float8e4

SK = 16.0            # host-side k scale into E4M3 range
SQ = 64.0            # device-side QM scale into E4M3 range
SQ2 = 16.0           # host-side q scale into E4M3 range

_NC_CACHE = {}

_SENT = object()


def _interleave(a_gen, b_gen, ratio_a=2):
    """Drain both generators; ratio_a steps of a per 1 of b while live."""
    a_live = b_live = True
    while a_live or b_live:
        if a_live:
            for _ in range(ratio_a):
                if next(a_gen, _SENT) is _SENT:
                    a_live = False
                    break
        if b_live and next(b_gen, _SENT) is _SENT:
            b_live = False


def _build_nc():
    nc = bacc.Bacc("TRN2", target_bir_lowering=False, debug=False,
                   num_devices=NCORES)

    qT8 = nc.dram_tensor("qT8", [D, CH], FP8, kind="ExternalInput")
    kT8 = nc.dram_tensor("kT8", [D, S], FP8, kind="ExternalInput")
    vn = nc.dram_tensor("vn", [S, D], BF16, kind="ExternalInput")
    wm = nc.dram_tensor("wm", [H, D, D], FP8, kind="ExternalInput")
    w2 = nc.dram_tensor("w2", [H, D, D], BF16, kind="ExternalInput")
    uv = nc.dram_tensor("uv", [H, D], F32, kind="ExternalInput")
    qmsc = nc.dram_tensor("qmsc", [P, H], F32, kind="ExternalInput")
    onesinv = nc.dram_tensor("onesinv", [P, 2], BF16, kind="ExternalInput")
    out = nc.dram_tensor("out", [CH, D], BF16, kind="ExternalOutput")

    with TileContext(nc) as tc:
        with (
            tc.tile_pool(name="consts", bufs=1) as consts,
            tc.tile_pool(name="qts", bufs=2) as qts,
            tc.tile_pool(name="pts", bufs=2) as pts,
            tc.tile_pool(name="small", bufs=3) as small,
            tc.tile_pool(name="accs", bufs=2) as accs,
            tc.tile_pool(name="ostage", bufs=4) as ostage,
            tc.tile_pool(name="rot", bufs=4, space="PSUM") as rot,
            tc.tile_pool(name="psout", bufs=1, space="PSUM") as psout,
        ):
            # ---- PE warm-up: tiny dummy matmuls keep the PE busy through
            # the initial DMA wait so the clock is ramped when the first
            # real matmul issues
            warm = consts.tile([P, 16], BF16, name="warm")
            nc.vector.memset(warm[:], 1.0)
            wps = rot.tile([P, 512], F32, tag="ps")
            for _i in range(96):
                nc.tensor.matmul(wps[0:16, 0:16], lhsT=warm[:],
                                 rhs=warm[:, 0:16], start=True, stop=True)

            # ---- SBUF-resident tensors, loaded in first-use order.
            wm_sb = consts.tile([P, H, DT, D], FP8, name="wm_sb")
            w2_sb = consts.tile([P, H, DT, D], BF16, name="w2_sb")
            q_sb = consts.tile([P, DT, CH], FP8, name="q_sb")
            k_sb = consts.tile([P, DT, S], FP8, name="k_sb")
            v_sb = consts.tile([P, KT, D], BF16, name="v_sb")
            u_sb = consts.tile([P, H, DT], F32, name="u_sb")
            qmsc_sb = consts.tile([P, H], F32, name="qmsc_sb")
            oinv_sb = consts.tile([P, 2], BF16, name="oinv_sb")
            acc = consts.tile([P, DT, CH], F32, name="acc")

            wm_ap = wm[:].rearrange("h (dt p) e -> p h dt e", p=P)
            w2_ap = w2[:].rearrange("h (dt p) e -> p h dt e", p=P)
            kap = kT8[:].rearrange("(dt p) s -> p dt s", p=P)
            vap = vn[:].rearrange("(kt p) d -> p kt d", p=P)
            qap = qT8[:].rearrange("(dt p) m -> p dt m", p=P)

            # loads split into ~128-256KB pieces issued in first-use order,
            # round-robined over the three DMA-capable engines so they
            # spread across many HW queues and land just-in-time
            dmae = [nc.sync, nc.scalar, nc.gpsimd]
            _rr = [0]

            def ld(dst, src):
                dmae[_rr[0] % 3].dma_start(dst, src)
                _rr[0] += 1

            # head-0 QM needs q + wm[0]; scores stream k tile-by-tile
            ld(q_sb[:, 0:2], qap[:, 0:2])
            ld(q_sb[:, 2:4], qap[:, 2:4])
            ld(wm_sb[:, 0, :, 0:256], wm_ap[:, 0, :, 0:256])
            ld(wm_sb[:, 0, :, 256:512], wm_ap[:, 0, :, 256:512])
            nc.gpsimd.dma_start(u_sb[:],
                                uv[:].rearrange("h (t p) -> p h t", p=P))
            nc.gpsimd.dma_start(qmsc_sb[:], qmsc[:])
            nc.gpsimd.dma_start(oinv_sb[:], onesinv[:])
            for i in range(8):
                ld(k_sb[:, :, i * 256:(i + 1) * 256],
                   kap[:, :, i * 256:(i + 1) * 256])
            for i in range(4):
                ld(v_sb[:, 2 * i:2 * i + 2], vap[:, 2 * i:2 * i + 2])
            ld(wm_sb[:, 1], wm_ap[:, 1])
            ld(w2_sb[:, 0, :, 0:256], w2_ap[:, 0, :, 0:256])
            ld(w2_sb[:, 0, :, 256:512], w2_ap[:, 0, :, 256:512])
            for i in range(4, 8):
                ld(v_sb[:, 2 * i:2 * i + 2], vap[:, 2 * i:2 * i + 2])
            for h in range(1, H):
                if h > 1:
                    ld(wm_sb[:, h], wm_ap[:, h])
                ld(w2_sb[:, h, :, 0:256], w2_ap[:, h, :, 0:256])
                ld(w2_sb[:, h, :, 256:512], w2_ap[:, h, :, 256:512])

            state = {}

            def qs_gen(h):
                """QM projection + scores + exp for head h (yields per MM)."""
                QTc = qts.tile([P, DT, CH], FP8, tag="QT")
                PT = pts.tile([P, KT, CH], BF16, tag="PT")
                state[h] = (QTc, PT)
                for et in range(DT):
                    ps = rot.tile([P, CH], F32, tag="ps")
                    for bq in range(2):
                        nc.tensor.matmul(
                            ps[:],
                            lhsT=wm_sb[:, h, 2 * bq:2 * bq + 2,
                                       et * P:(et + 1) * P],
                            rhs=q_sb[:, 2 * bq:2 * bq + 2, :],
                            start=(bq == 0), stop=(bq == 1),
                            perf_mode=mybir.MatmulPerfMode.DoubleRow,
                        )
                        yield
                    # QTc = (ps + u*sw*SQ2) * (SQ/(sw*SQ2)), per-head scale
                    nc.vector.tensor_scalar(
                        QTc[:, et, :], ps[:],
                        u_sb[:, h, et:et + 1], qmsc_sb[:, h:h + 1],
                        mybir.AluOpType.add, mybir.AluOpType.mult,
                    )
                for kt in range(KT):
                    ps = rot.tile([P, CH], F32, tag="ps")
                    for bk in range(2):
                        nc.tensor.matmul(
                            ps[:],
                            lhsT=k_sb[:, 2 * bk:2 * bk + 2,
                                      kt * P:(kt + 1) * P],
                            rhs=QTc[:, 2 * bk:2 * bk + 2, :],
                            start=(bk == 0), stop=(bk == 1),
                            perf_mode=mybir.MatmulPerfMode.DoubleRow,
                        )
                        yield
                    nc.scalar.activation(
                        PT[:, kt, :], ps[:],
                        mybir.ActivationFunctionType.Exp,
                        scale=1.0 / (SK * SQ),
                    )

            def av_tail(h):
                """AV + denominator + out-projection + accumulate, head h."""
                QTc, PT = state.pop(h)

                outT_ps = psout.tile([P, DT, CH], F32, tag="outT")
                AT_sb = small.tile([P, DT, CH], BF16, tag="AT")
                denA = small.tile([P, CH], F32, tag="denA")
                denBc = small.tile([P, CH], F32, tag="denBc")
                denB_sb = small.tile([P, CH], BF16, tag="denB_sb")
                for et in range(DT):
                    for kt in range(KT):
                        nc.tensor.matmul(
                            outT_ps[:, et, :],
                            lhsT=v_sb[:, kt, et * P:(et + 1) * P],
                            rhs=PT[:, kt, :],
                            start=(kt == 0), stop=(kt == KT - 1),
                        )
                        yield
                    nc.vector.tensor_copy(AT_sb[:, et, :], outT_ps[:, et, :])
                    if et == 0:
                        # denominator: two add chains over PT tiles on the
                        # otherwise-idle GpSimd engine (keeps Vector free
                        # for QT casts / AT copies / the accumulator)
                        nc.gpsimd.tensor_add(denA[:], PT[:, 0, :],
                                             PT[:, 2, :])
                        nc.gpsimd.tensor_add(denBc[:], PT[:, 1, :],
                                             PT[:, 3, :])
                        for kt in range(4, KT, 2):
                            nc.gpsimd.tensor_add(denA[:], denA[:],
                                                 PT[:, kt, :])
                            nc.gpsimd.tensor_add(denBc[:], denBc[:],
                                                 PT[:, kt + 1, :])
                        nc.gpsimd.tensor_add(denB_sb[:], denA[:], denBc[:])
                denT_ps = rot.tile([P, CH], F32, tag="ps")
                for t in range(4):
                    nc.tensor.matmul(
                        denT_ps[:, 2 * t:2 * t + 2],
                        lhsT=denB_sb[:, t * P:(t + 1) * P],
                        rhs=oinv_sb[:],
                        start=True, stop=True,
                    )
                yield
                recipT = small.tile([P, 8], F32, tag="recipT")
                nc.vector.reciprocal(recipT[:], denT_ps[:, 0:8])
                # out-projection into the f32 accumulator (sum over heads)
                for t in range(4):
                    ps = rot.tile([P, CH], F32, tag="ps")
                    for et in range(DT):
                        nc.tensor.matmul(
                            ps[:],
                            lhsT=AT_sb[:, et, t * P:(t + 1) * P],
                            rhs=w2_sb[:, h, et, :],
                            start=(et == 0), stop=(et == DT - 1),
                        )
                        yield
                    if h == 0:
                        nc.vector.tensor_scalar_mul(
                            acc[:, t, :], ps[:], recipT[:, 2 * t:2 * t + 1]
                        )
                    else:
                        sc = accs.tile([P, CH], F32, tag="sc")
                        nc.vector.tensor_scalar_mul(
                            sc[:], ps[:], recipT[:, 2 * t:2 * t + 1]
                        )
                        if h < H - 1:
                            nc.vector.tensor_add(acc[:, t, :], acc[:, t, :],
                                                 sc[:])
                        else:
                            o_sb = ostage.tile([P, CH], BF16, tag="o")
                            nc.vector.tensor_add(o_sb[:], acc[:, t, :],
                                                 sc[:])
                            eng = nc.sync if t % 2 == 0 else nc.scalar
                            eng.dma_start(out[t * P:(t + 1) * P, :], o_sb[:])

            # ---- software pipeline: QM+scores(h+1) hides inside AV(h)
            prev_tail = None
            for h in range(H):
                qs = qs_gen(h)
                if prev_tail is None:
                    for _ in qs:
                        pass
                else:
                    _interleave(prev_tail, qs, ratio_a=2)
                prev_tail = av_tail(h)
            for _ in prev_tail:
                pass

    nc.compile()
    return nc


def kernel(q, k, v, Wq, Wk, Wv, bq, bk, bv, Wo, bo):
    import ml_dtypes

    if "nc" not in _NC_CACHE:
        _NC_CACHE["nc"] = _build_nc()
    nc = _NC_CACHE["nc"]

    q = np.asarray(q, dtype=np.float32)
    k = np.asarray(k, dtype=np.float32)
    v = np.asarray(v, dtype=np.float32)
    Wq = np.asarray(Wq, dtype=np.float32)
    Wk = np.asarray(Wk, dtype=np.float32)
    Wv = np.asarray(Wv, dtype=np.float32)
    bq = np.asarray(bq, dtype=np.float32)
    bv = np.asarray(bv, dtype=np.float32)
    Wo = np.asarray(Wo, dtype=np.float32)
    bo = np.asarray(bo, dtype=np.float32)

    def cast16(x):
        return np.ascontiguousarray(
            np.asarray(x, dtype=np.float32).astype(ml_dtypes.bfloat16))

    def cast8(x, s):
        return np.ascontiguousarray(
            np.clip(np.asarray(x, np.float32) * s, -240.0, 240.0)
            .astype(ml_dtypes.float8_e4m3))

    scale = np.float32(1.0 / np.sqrt(D))

    # shared (replicated) weights
    wm_all = np.empty((H, D, D), dtype=ml_dtypes.float8_e4m3)
    w2_all = np.empty((H, D, D), dtype=ml_dtypes.bfloat16)
    uv_all = np.empty((H, D), dtype=np.float32)
    qmsc_all = np.empty((P, H), dtype=np.float32)
    for h in range(H):
        Wo_h = Wo[h * D:(h + 1) * D, :]
        wm_f = (Wq[h] * scale) @ Wk[h].T
        u_f = (bq[h] * scale) @ Wk[h].T
        # per-head power-of-2 weight scale into E4M3's normal range
        sw = float(2.0 ** np.floor(np.log2(
            128.0 / max(np.abs(wm_f).max(), 1e-30))))
        wm_all[h] = cast8(wm_f, sw)
        w2_all[h] = cast16(Wv[h] @ Wo_h)
        uv_all[h] = u_f * (sw * SQ2)
        qmsc_all[:, h] = SQ / (sw * SQ2)
    onesinv = cast16(np.ones((P, 2), dtype=np.float32))

    in_maps = []
    for c in range(NCORES):
        b, qc = divmod(c, MC)
        in_maps.append({
            "qT8": cast8(q[b].T[:, qc * CH:(qc + 1) * CH], SQ2),
            "kT8": cast8(k[b].T, SK),
            "vn": cast16(v[b]),
            "wm": wm_all, "w2": w2_all, "uv": uv_all, "qmsc": qmsc_all,
            "onesinv": onesinv,
        })

    trace = bool(int(os.environ.get("KERNEL_TRACE", "0")))
    res = bass_utils.run_bass_kernel_spmd(
        nc, in_maps, core_ids=list(range(NCORES)), trace=trace
    )
    _NC_CACHE["last_result"] = res

    c_const = (sum(bv[h] @ Wo[h * D:(h + 1) * D, :] for h in range(H))
               + bo).astype(np.float32)
    out = np.empty((B, S, D), dtype=np.float32)
    for c in range(NCORES):
        b, qc = divmod(c, MC)
        out[b, qc * CH:(qc + 1) * CH, :] = (
            np.asarray(res.results[c]["out"], dtype=np.float32) + c_const)
    return out


# revision 7
# speedup vs baseline: 1.0566x; 1.0566x over previous
"""Query-chunk-parallel MultiHeadAttention kernel for 8 Trainium2 cores.

Problem: B=2, S=2048, D=512, H=8, per-head full-width projections.

Sharding: the B*S=4096 query rows split into 8 chunks of 512; chunk c
-> core c (b = c//4). Each core computes ALL 8 heads for its 512 query
rows and writes its own [512, D] slice of the final output: ZERO
collectives (the head-parallel variant spent ~70us in an exposed
ReduceScatter/AllGather tail plus a saturated cc stream).

Math restructuring (inherited from the verified head-parallel kernel):
  - softmax row-equivalences drop the K bias bk entirely; the V bias bv
    reduces to a constant row c = sum_h bv[h] @ Wo_h + bo added on the
    host at the end.
  - Host-fused weights (weight-weight products only):
      M_h = (Wq[h]/sqrt(D)) @ Wk[h]^T   so scores = q M_h k^T
      u_h = (bq[h]/sqrt(D)) @ Wk[h]^T   per-partition bias on QM^T
      W2_h = Wv[h] @ Wo_h               so out += (attn @ v) @ W2_h / den
    This removes the on-device K and V projections completely.
  - No softmax max-subtraction: score std ~0.33, |scores| < ~2.5.

Dataflow per head h (on this core's 512-row query chunk):
  QM^T[d2,m] = M_h^T q^T, +u, *64 -> fp8   (8 MM, fp8 DoubleRow)
  sT[k,m]    = k8 QM8                      (32 MM, fp8 DoubleRow)
               -> exp(ps/1024) on ACT -> PT (bf16)
  AT[d,m]    = v^T P (bf16)                (64 MM), den = DVE adds
  acc[m,e]  += (AT^T W2_h) / den_h         (16 MM + 2 DVE)
After head 7: acc (f32) -> bf16 -> DMA to the core's out slice.

Perf notes (from NTFF traces of the head-parallel ancestor):
  - Every 128-part x 512-free matmul costs ~263ns regardless of dtype
    (512 rows at the sustained-clock rate); fp8 DoubleRow wins by
    contracting 256 rows/MM, i.e. halved MM count, not faster MMs.
    960 MMs/core ~= 253us is the PE floor at these precisions.
  - Software pipelining: QM+scores of head h+1 interleave into the
    AV/out-projection of head h on the PE (ratio 2:1) so the exp (ACT)
    latency and QM->fp8 casts (DVE) never pace the tensor engine.
  - PE warm-up dummy matmuls keep the clock ramped through the initial
    DMA wait; loads are ordered first-use-first (wm_h0+q, k, v, ...).
"""
import os
import sys

sys.path.insert(0, "/opt/trn_rl_repo")
sys.path.insert(0, "/root/.axon_site")

import numpy as np

import concourse.bacc as bacc
import concourse.mybir as mybir
from concourse.tile import TileContext
from concourse import bass_utils

P = 128
B, S, D, H = 2, 2048, 512, 8
NCORES = 8
MC = 4               # query chunks per batch; B*MC == NCORES
CH = S // MC         # 512 query rows per core
DT = D // P          # 4 feature tiles
KT = S // P          # 16 k tiles (full batch seq per core)
F32 = mybir.dt.float32
BF16 = mybir.dt.bfloat16
FP8 = mybir.dt.float8e4

SK = 16.0            # host-side k scale into E4M3 range
SQ = 64.0            # device-side QM scale into E4M3 range
SQ2 = 16.0           # host-side q scale into E4M3 range

_NC_CACHE = {}

_SENT = object()


def _interleave(a_gen, b_gen, ratio_a=2):
    """Drain both generators; ratio_a steps of a per 1 of b while live."""
    a_live = b_live = True
    while a_live or b_live:
        if a_live:
            for _ in range(ratio_a):
                if next(a_gen, _SENT) is _SENT:
                    a_live = False
                    break
        if b_live and next(b_gen, _SENT) is _SENT:
            b_live = False


def _build_nc():
    nc = bacc.Bacc("TRN2", target_bir_lowering=False, debug=False,
                   num_devices=NCORES)

    qT8 = nc.dram_tensor("qT8", [D, CH], FP8, kind="ExternalInput")
    kT8 = nc.dram_tensor("kT8", [D, S], FP8, kind="ExternalInput")
    vn = nc.dram_tensor("vn", [S, D], BF16, kind="ExternalInput")
    wm = nc.dram_tensor("wm", [H, D, D], FP8, kind="ExternalInput")
    w2 = nc.dram_tensor("w2", [H, D, D], BF16, kind="ExternalInput")
    uv = nc.dram_tensor("uv", [H, D], F32, kind="ExternalInput")
    qmsc = nc.dram_tensor("qmsc", [P, H], F32, kind="ExternalInput")
    onesinv = nc.dram_tensor("onesinv", [P, 2], BF16, kind="ExternalInput")
    out = nc.dram_tensor("out", [CH, D], BF16, kind="ExternalOutput")

    with TileContext(nc) as tc:
        with (
            tc.tile_pool(name="consts", bufs=1) as consts,
            tc.tile_pool(name="qts", bufs=2) as qts,
            tc.tile_pool(name="pts", bufs=2) as pts,
            tc.tile_pool(name="small", bufs=3) as small,
            tc.tile_pool(name="accs", bufs=2) as accs,
            tc.tile_pool(name="ostage", bufs=4) as ostage,
            tc.tile_pool(name="rot", bufs=4, space="PSUM") as rot,
            tc.tile_pool(name="psout", bufs=1, space="PSUM") as psout,
        ):
            # ---- PE warm-up: tiny dummy matmuls keep the PE busy through
            # the initial DMA wait so the clock is ramped when the first
            # real matmul issues
            warm = consts.tile([P, 16], BF16, name="warm")
            nc.vector.memset(warm[:], 1.0)
            wps = rot.tile([P, 512], F32, tag="ps")
            for _i in range(96):
                nc.tensor.matmul(wps[0:16, 0:16], lhsT=warm[:],
                                 rhs=warm[:, 0:16], start=True, stop=True)

            # ---- SBUF-resident tensors, loaded in first-use order.
            wm_sb = consts.tile([P, H, DT, D], FP8, name="wm_sb")
            w2_sb = consts.tile([P, H, DT, D], BF16, name="w2_sb")
            q_sb = consts.tile([P, DT, CH], FP8, name="q_sb")
            k_sb = consts.tile([P, DT, S], FP8, name="k_sb")
            v_sb = consts.tile([P, KT, D], BF16, name="v_sb")
            u_sb = consts.tile([P, H, DT], F32, name="u_sb")
            qmsc_sb = consts.tile([P, H], F32, name="qmsc_sb")
            oinv_sb = consts.tile([P, 2], BF16, name="oinv_sb")
            acc = consts.tile([P, DT, CH], F32, name="acc")

            wm_ap = wm[:].rearrange("h (dt p) e -> p h dt e", p=P)
            w2_ap = w2[:].rearrange("h (dt p) e -> p h dt e", p=P)
            kap = kT8[:].rearrange("(dt p) s -> p dt s", p=P)
            vap = vn[:].rearrange("(kt p) d -> p kt d", p=P)
            qap = qT8[:].rearrange("(dt p) m -> p dt m", p=P)

            # three just-in-time DMA streams (sync / scalar / gpsimd):
            # per-engine streams drain in order at ~35-65 GB/s with a few
            # us of per-call issue overhead, so keep calls coarse and put
            # each tensor piece on the stream that gets it there by the
            # time compute needs it (QM ~7us: q+wm0; scores ~12us+: k
            # tiles in order; AV ~24us+: v tiles in order; out-proj(0)
            # ~32us: w2_0; QS(1) ~28us: wm1).
            nc.sync.dma_start(wm_sb[:, 0], wm_ap[:, 0])
            nc.scalar.dma_start(q_sb[:], qap[:])
            nc.gpsimd.dma_start(u_sb[:],
                                uv[:].rearrange("h (t p) -> p h t", p=P))
            nc.gpsimd.dma_start(qmsc_sb[:], qmsc[:])
            nc.gpsimd.dma_start(oinv_sb[:], onesinv[:])
            # k: tiles 0-7 / 8-11 / 12-15 land in consumption order
            nc.sync.dma_start(k_sb[:, :, 0:1024], kap[:, :, 0:1024])
            nc.scalar.dma_start(k_sb[:, :, 1024:1536], kap[:, :, 1024:1536])
            nc.gpsimd.dma_start(k_sb[:, :, 1536:2048], kap[:, :, 1536:2048])
            # v: tiles 0-5 / 6-10 / 11-15 in consumption order
            nc.sync.dma_start(v_sb[:, 0:6], vap[:, 0:6])
            nc.scalar.dma_start(v_sb[:, 6:11], vap[:, 6:11])
            nc.gpsimd.dma_start(v_sb[:, 11:16], vap[:, 11:16])
            nc.sync.dma_start(wm_sb[:, 1], wm_ap[:, 1])
            nc.scalar.dma_start(w2_sb[:, 0], w2_ap[:, 0])
            nc.gpsimd.dma_start(wm_sb[:, 2], wm_ap[:, 2])
            nc.sync.dma_start(w2_sb[:, 1], w2_ap[:, 1])
            nc.scalar.dma_start(wm_sb[:, 3], wm_ap[:, 3])
            nc.gpsimd.dma_start(w2_sb[:, 2], w2_ap[:, 2])
            for h in range(4, H):
                nc.sync.dma_start(wm_sb[:, h], wm_ap[:, h])
            for h in range(3, H):
                (nc.scalar if h % 2 else nc.gpsimd).dma_start(
                    w2_sb[:, h], w2_ap[:, h])

            state = {}

            def qs_gen(h):
                """QM projection + scores + exp for head h (yields per MM)."""
                QTc = qts.tile([P, DT, CH], FP8, tag="QT")
                PT = pts.tile([P, KT, CH], BF16, tag="PT")
                state[h] = (QTc, PT)
                for et in range(DT):
                    ps = rot.tile([P, CH], F32, tag="ps")
                    for bq in range(2):
                        nc.tensor.matmul(
                            ps[:],
                            lhsT=wm_sb[:, h, 2 * bq:2 * bq + 2,
                                       et * P:(et + 1) * P],
                            rhs=q_sb[:, 2 * bq:2 * bq + 2, :],
                            start=(bq == 0), stop=(bq == 1),
                            perf_mode=mybir.MatmulPerfMode.DoubleRow,
                        )
                        yield
                    # QTc = (ps + u*sw*SQ2) * (SQ/(sw*SQ2)), per-head scale
                    nc.vector.tensor_scalar(
                        QTc[:, et, :], ps[:],
                        u_sb[:, h, et:et + 1], qmsc_sb[:, h:h + 1],
                        mybir.AluOpType.add, mybir.AluOpType.mult,
                    )
                for kt in range(KT):
                    ps = rot.tile([P, CH], F32, tag="ps")
                    for bk in range(2):
                        nc.tensor.matmul(
                            ps[:],
                            lhsT=k_sb[:, 2 * bk:2 * bk + 2,
                                      kt * P:(kt + 1) * P],
                            rhs=QTc[:, 2 * bk:2 * bk + 2, :],
                            start=(bk == 0), stop=(bk == 1),
                            perf_mode=mybir.MatmulPerfMode.DoubleRow,
                        )
                        yield
                    nc.scalar.activation(
                        PT[:, kt, :], ps[:],
                        mybir.ActivationFunctionType.Exp,
                        scale=1.0 / (SK * SQ),
                    )

            def av_tail(h):
                """AV + denominator + out-projection + accumulate, head h."""
                QTc, PT = state.pop(h)

                outT_ps = psout.tile([P, DT, CH], F32, tag="outT")
                AT_sb = small.tile([P, DT, CH], BF16, tag="AT")
                denA = small.tile([P, CH], F32, tag="denA")
                denBc = small.tile([P, CH], F32, tag="denBc")
                denB_sb = small.tile([P, CH], BF16, tag="denB_sb")
                for et in range(DT):
                    for kt in range(KT):
                        nc.tensor.matmul(
                            outT_ps[:, et, :],
                            lhsT=v_sb[:, kt, et * P:(et + 1) * P],
                            rhs=PT[:, kt, :],
                            start=(kt == 0), stop=(kt == KT - 1),
                        )
                        yield
                    nc.vector.tensor_copy(AT_sb[:, et, :], outT_ps[:, et, :])
                    if et == 0:
                        # denominator: two DVE chains over PT tiles, after
                        # the first AT cast so scores(h+1) aren't gated
                        nc.vector.tensor_add(denA[:], PT[:, 0, :],
                                             PT[:, 2, :])
                        nc.vector.tensor_add(denBc[:], PT[:, 1, :],
                                             PT[:, 3, :])
                        for kt in range(4, KT, 2):
                            nc.vector.tensor_add(denA[:], denA[:],
                                                 PT[:, kt, :])
                            nc.vector.tensor_add(denBc[:], denBc[:],
                                                 PT[:, kt + 1, :])
                        nc.vector.tensor_add(denB_sb[:], denA[:], denBc[:])
                denT_ps = rot.tile([P, CH], F32, tag="ps")
                for t in range(4):
                    nc.tensor.matmul(
                        denT_ps[:, 2 * t:2 * t + 2],
                        lhsT=denB_sb[:, t * P:(t + 1) * P],
                        rhs=oinv_sb[:],
                        start=True, stop=True,
                    )
                yield
                recipT = small.tile([P, 8], F32, tag="recipT")
                nc.vector.reciprocal(recipT[:], denT_ps[:, 0:8])
                # out-projection into the f32 accumulator (sum over heads)
                for t in range(4):
                    ps = rot.tile([P, CH], F32, tag="ps")
                    for et in range(DT):
                        nc.tensor.matmul(
                            ps[:],
                            lhsT=AT_sb[:, et, t * P:(t + 1) * P],
                            rhs=w2_sb[:, h, et, :],
                            start=(et == 0), stop=(et == DT - 1),
                        )
                        yield
                    if h == 0:
                        nc.vector.tensor_scalar_mul(
                            acc[:, t, :], ps[:], recipT[:, 2 * t:2 * t + 1]
                        )
                    else:
                        sc = accs.tile([P, CH], F32, tag="sc")
                        nc.vector.tensor_scalar_mul(
                            sc[:], ps[:], recipT[:, 2 * t:2 * t + 1]
                        )
                        if h < H - 1:
                            nc.vector.tensor_add(acc[:, t, :], acc[:, t, :],
                                                 sc[:])
                        else:
                            o_sb = ostage.tile([P, CH], BF16, tag="o")
                            nc.vector.tensor_add(o_sb[:], acc[:, t, :],
                                                 sc[:])
                            eng = nc.sync if t % 2 == 0 else nc.scalar
                            eng.dma_start(out[t * P:(t + 1) * P, :], o_sb[:])

            # ---- software pipeline: QM+scores(h+1) hides inside AV(h)
            prev_tail = None
            for h in range(H):
                qs = qs_gen(h)
                if prev_tail is None:
                    for _ in qs:
                        pass
                else:
                    _interleave(prev_tail, qs, ratio_a=2)
                prev_tail = av_tail(h)
            for _ in prev_tail:
                pass

    nc.compile()
    return nc


def kernel(q, k, v, Wq, Wk, Wv, bq, bk, bv, Wo, bo):
    import ml_dtypes

    if "nc" not in _NC_CACHE:
        _NC_CACHE["nc"] = _build_nc()
    nc = _NC_CACHE["nc"]

    q = np.asarray(q, dtype=np.float32)
    k = np.asarray(k, dtype=np.float32)
    v = np.asarray(v, dtype=np.float32)
    Wq = np.asarray(Wq, dtype=np.float32)
    Wk = np.asarray(Wk, dtype=np.float32)
    Wv = np.asarray(Wv, dtype=np.float32)
    bq = np.asarray(bq, dtype=np.float32)
    bv = np.asarray(bv, dtype=np.float32)
    Wo = np.asarray(Wo, dtype=np.float32)
    bo = np.asarray(bo, dtype=np.float32)

    def cast16(x):
        return np.ascontiguousarray(
            np.asarray(x, dtype=np.float32).astype(ml_dtypes.bfloat16))

    def cast8(x, s):
        return np.ascontiguousarray(
            np.clip(np.asarray(x, np.float32) * s, -240.0, 240.0)
            .astype(ml_dtypes.float8_e4m3))

    scale = np.float32(1.0 / np.sqrt(D))

    # shared (replicated) weights
    wm_all = np.empty((H, D, D), dtype=ml_dtypes.float8_e4m3)
    w2_all = np.empty((H, D, D), dtype=ml_dtypes.bfloat16)
    uv_all = np.empty((H, D), dtype=np.float32)
    qmsc_all = np.empty((P, H), dtype=np.float32)
    for h in range(H):
        Wo_h = Wo[h * D:(h + 1) * D, :]
        wm_f = (Wq[h] * scale) @ Wk[h].T
        u_f = (bq[h] * scale) @ Wk[h].T
        # per-head power-of-2 weight scale into E4M3's normal range
        sw = float(2.0 ** np.floor(np.log2(
            128.0 / max(np.abs(wm_f).max(), 1e-30))))
        wm_all[h] = cast8(wm_f, sw)
        w2_all[h] = cast16(Wv[h] @ Wo_h)
        uv_all[h] = u_f * (sw * SQ2)
        qmsc_all[:, h] = SQ / (sw * SQ2)
    onesinv = cast16(np.ones((P, 2), dtype=np.float32))

    in_maps = []
    for c in range(NCORES):
        b, qc = divmod(c, MC)
        in_maps.append({
            "qT8": cast8(q[b].T[:, qc * CH:(qc + 1) * CH], SQ2),
            "kT8": cast8(k[b].T, SK),
            "vn": cast16(v[b]),
            "wm": wm_all, "w2": w2_all, "uv": uv_all, "qmsc": qmsc_all,
            "onesinv": onesinv,
        })

    trace = bool(int(os.environ.get("KERNEL_TRACE", "0")))
    res = bass_utils.run_bass_kernel_spmd(
        nc, in_maps, core_ids=list(range(NCORES)), trace=trace
    )
    _NC_CACHE["last_result"] = res

    c_const = (sum(bv[h] @ Wo[h * D:(h + 1) * D, :] for h in range(H))
               + bo).astype(np.float32)
    out = np.empty((B, S, D), dtype=np.float32)
    for c in range(NCORES):
        b, qc = divmod(c, MC)
        out[b, qc * CH:(qc + 1) * CH, :] = (
            np.asarray(res.results[c]["out"], dtype=np.float32) + c_const)
    return out
